# revision 54
# baseline (speedup 1.0000x reference)
"""Trainium2 Bass kernel for nn_Contextualizer (sparse_attention).

Per-core computation (data-parallel over batch B=8 across 8 NeuronCores):
    x0, x1 = split(x, 2, axis=-1)            # [N, D] each, N=2048, D=1024
    xn = x0 / sqrt(sum(x0^2, -1) + eps)      # row-normalize
    cosim = xn @ xn.T                        # [N, N], symmetric
    attn = tril(spatial_proj) * cosim
    out = (attn @ x0) * x1

Gram formulation with both 1/norm factors folded into elementwise stages:
    A[m, n]   = maskT[m, n] * G[m, n] * s[m]     (mask stage, m on partitions)
    out[n, d] = (ctx'[n, d] * s[n]) * x1[n, d]   (gating stage)
where s = 1/sqrt(sum x0^2 + eps), maskT = tril(spatial_proj).T (host-side,
bf16), and A (= attn transposed, via G symmetry) feeds matmul2 as the
stationary operand: ctx'[n, d] = sum_m A[m, n] * x0[m, d].

The tril mask kills the upper triangle: matmul1 strips are 512 wide with
partial-width matmuls at the diagonal (exact triangular work), matmul2
accumulates only tiles m <= n.  PE transposes of x0 (for the Gram moving
operand) are dripped 1:1 between strip/ctx matmuls so their identity
LDWEIGHTS hide under the 213ns matmuls.
"""

import numpy as np

B = 8
N = 2048
D = 1024
P = 128
NT = N // P      # 16 row tiles
DK = D // P      # 8 contraction chunks for matmul1
SW = 512         # matmul1 strip width (n)
NS = N // SW     # 4 strips
TPS = SW // P    # 4 row tiles per strip
EPS = 1e-8

_NC_CACHE = {}


def _build():
    from concourse import bacc, mybir
    from concourse.tile import TileContext
    from concourse.masks import make_identity

    f32 = mybir.dt.float32
    f32r = mybir.dt.float32r
    bf16 = mybir.dt.bfloat16
    AF = mybir.ActivationFunctionType
    OP = mybir.AluOpType

    nc = bacc.Bacc("TRN2", debug=False, num_devices=B)
    x0_ext = nc.declare_dram_parameter("x0b", [N, D], bf16, isOutput=False)
    x1_ext = nc.declare_dram_parameter("x1b", [N, D], bf16, isOutput=False)
    m_ext = nc.declare_dram_parameter("maskT", [N, N], bf16, isOutput=False)
    out_ext = nc.declare_dram_parameter("out", [N, D], bf16, isOutput=True)

    with TileContext(nc) as tc:
        with (
            tc.tile_pool(name="big", bufs=1) as big,
            tc.tile_pool(name="sqp", bufs=2) as sqp,
            tc.tile_pool(name="astrip", bufs=3) as astrip,
            tc.tile_pool(name="maskp", bufs=14) as maskp,
            tc.tile_pool(name="gio", bufs=4) as gio,
            tc.tile_pool(name="outp", bufs=2) as outp,
            tc.tile_pool(name="pt", bufs=2, space="PSUM") as pt,
            tc.tile_pool(name="pa", bufs=3, space="PSUM") as pa,
            tc.tile_pool(name="pb", bufs=3, space="PSUM") as pb,
        ):
            xb = big.tile([P, NT, D], bf16)    # x0 bf16, natural [m, d]
            x0T = big.tile([P, DK, N], bf16)   # x0 transposed: [d, n]
            ident = big.tile([P, P], bf16)
            make_identity(nc, ident)
            nrm2 = big.tile([P, NT], f32)
            scal = big.tile([P, NT], f32)
            tmpa = big.tile([P, NT], f32)
            tmpb = big.tile([P, NT], f32)

            def load_tile(i, split=False):
                """DMA x0 tile i (bf16, host-cast) straight into xb."""
                src = x0_ext.ap()[i * P : (i + 1) * P, :]
                if split:
                    # Halves on both queues: transposes of e<4 start as soon
                    # as the first half lands.
                    nc.sync.dma_start(xb[:, i, 0:512], src[:, 0:512])
                    nc.scalar.dma_start(xb[:, i, 512:D], src[:, 512:D])
                else:
                    eng = nc.sync if i % 2 == 0 else nc.scalar
                    eng.dma_start(xb[:, i, :], src[:])

            def square_tile(i):
                sq = sqp.tile([P, D], f32, tag="sq")
                nc.scalar.activation(
                    sq[:], xb[:, i, :], AF.Square, accum_out=nrm2[:, i : i + 1]
                )

            def stats_range(i0, i1):
                """scal[:, i0:i1+1] = rsqrt(nrm2 + EPS), one Newton step."""
                sl = slice(i0, i1 + 1)
                nc.gpsimd.tensor_scalar_add(tmpa[:, sl], nrm2[:, sl], EPS)
                nc.scalar.activation(tmpb[:, sl], tmpa[:, sl], AF.Sqrt)
                nc.vector.reciprocal(scal[:, sl], tmpb[:, sl])
                nc.gpsimd.tensor_mul(tmpb[:, sl], scal[:, sl], scal[:, sl])
                nc.gpsimd.tensor_mul(tmpb[:, sl], tmpb[:, sl], tmpa[:, sl])
                nc.gpsimd.tensor_scalar(
                    tmpb[:, sl], tmpb[:, sl], -0.5, 1.5, op0=OP.mult, op1=OP.add
                )
                nc.gpsimd.tensor_mul(scal[:, sl], scal[:, sl], tmpb[:, sl])

            # Pending PE-transpose drip queue: (tile, e) pairs in order.
            pend = []
            pt_cur = [None]

            def drip(k):
                """Emit up to k pending transpose matmuls (+copy at tile end)."""
                for _ in range(k):
                    if not pend:
                        return
                    i, e = pend.pop(0)
                    if e == 0:
                        pt_cur[0] = pt.tile(
                            [P, DK, P], bf16, tag="ps", name="ps"
                        )
                    ps = pt_cur[0]
                    nc.tensor.transpose(
                        ps[:, e, :], xb[:, i, e * P : (e + 1) * P], ident
                    )
                    if e == DK - 1:
                        nc.scalar.copy(x0T[:, :, i * P : (i + 1) * P], ps[:])

            mask_rr = [0]
            mask_q = [None] * NS

            def mask_load(nj):
                """Issue strip nj's mask DMAs (sync queue, ahead of use)."""
                n0 = nj * SW
                mts = []
                for mi2 in range(0, TPS * nj + TPS, 2):
                    mt = maskp.tile([P, 2, SW], bf16, tag="mt", name="mt")
                    # Late strips alternate sync/scalar so mask transfers
                    # stop queueing ahead of tile 8-15 loads on sync.
                    if nj >= 2:
                        eng = nc.sync if mask_rr[0] % 2 == 0 else nc.scalar
                    else:
                        eng = nc.sync
                    mask_rr[0] += 1
                    eng.dma_start(
                        mt[:],
                        m_ext.ap()[
                            mi2 * P : (mi2 + 2) * P, n0 : n0 + SW
                        ].rearrange("(c p) n -> p c n", p=P),
                    )
                    mts.append(mt)
                mask_q[nj] = mts

            def phase_a(nj):
                """A strip for n in [SW*nj, SW*nj+SW): m-tiles 0..4nj+3."""
                n0 = nj * SW
                n_mtiles = TPS * nj + TPS
                A = astrip.tile([P, NT, SW], bf16, tag="A")
                for mi2 in range(0, n_mtiles, 2):
                    mt = mask_q[nj][mi2 // 2]
                    for c in range(2):
                        mi = mi2 + c
                        off = max(0, (mi - TPS * nj) * P)
                        pcs = pa.tile([P, SW], f32, tag="pcs")
                        for e in range(DK):
                            nc.tensor.matmul(
                                pcs[:, off:SW],
                                x0T[:, e, mi * P : (mi + 1) * P],
                                x0T[:, e, n0 + off : n0 + SW],
                                start=(e == 0),
                                stop=(e == DK - 1),
                            )
                            # Defer dripping past the first pair so the next
                            # group's tiles have landed before their
                            # transposes hit the in-order PE queue.
                            if mi2 > 0:
                                drip(2)
                        nc.vector.scalar_tensor_tensor(
                            out=A[:, mi, off:SW],
                            in0=pcs[:, off:SW],
                            scalar=scal[:, mi : mi + 1],
                            in1=mt[:, c, off:SW],
                            op0=OP.mult,
                            op1=OP.mult,
                        )
                return A

            def phase_b(nj, A, last=False):
                """ctx rows for n-tiles 4nj..4nj+3; scale+gate; DMA out."""
                x1ts = []
                for sub in range(TPS):
                    ni = TPS * nj + sub
                    x1t = gio.tile([P, D], bf16, tag="x1t")
                    nc.gpsimd.dma_start(
                        x1t[:], x1_ext.ap()[ni * P : (ni + 1) * P, :]
                    )
                    x1ts.append(x1t)
                for sub in range(TPS):
                    ni = TPS * nj + sub
                    x1t = x1ts[sub]
                    ot = outp.tile([P, D], bf16, tag="ot")
                    if last and sub == TPS - 1:
                        # Tail: quarter-width PSUM groups so gating + the
                        # final store overlap the last matmuls.
                        for q in range(4):
                            pq = pb.tile([P, 512], f32, tag="pob", name="pob")
                            for mi in range(ni + 1):
                                nc.tensor.matmul(
                                    pq[:, 0:256],
                                    A[:, mi, sub * P : (sub + 1) * P],
                                    xb[:, mi, q * 256 : (q + 1) * 256],
                                    start=(mi == 0),
                                    stop=(mi == ni),
                                )
                            c0 = q * 256
                            nc.vector.scalar_tensor_tensor(
                                out=ot[:, c0 : c0 + 256],
                                in0=pq[:, 0:256],
                                scalar=scal[:, ni : ni + 1],
                                in1=x1t[:, c0 : c0 + 256],
                                op0=OP.mult,
                                op1=OP.mult,
                            )
                            eng = nc.scalar if q % 2 == 0 else nc.sync
                            eng.dma_start(
                                out_ext.ap()[
                                    ni * P : (ni + 1) * P, c0 : c0 + 256
                                ],
                                ot[:, c0 : c0 + 256],
                            )
                        continue
                    pob0 = pb.tile([P, 512], f32, tag="pob")
                    pob1 = pb.tile([P, 512], f32, tag="pob")
                    for mi in range(ni + 1):
                        st = A[:, mi, sub * P : (sub + 1) * P]
                        nc.tensor.matmul(
                            pob0[:], st, xb[:, mi, 0:512],
                            start=(mi == 0), stop=(mi == ni),
                        )
                        nc.tensor.matmul(
                            pob1[:], st, xb[:, mi, 512:D],
                            start=(mi == 0), stop=(mi == ni),
                        )
                        drip(1)
                    for dc, pob in enumerate((pob0, pob1)):
                        nc.vector.scalar_tensor_tensor(
                            out=ot[:, dc * 512 : (dc + 1) * 512],
                            in0=pob[:],
                            scalar=scal[:, ni : ni + 1],
                            in1=x1t[:, dc * 512 : (dc + 1) * 512],
                            op0=OP.mult,
                            op1=OP.mult,
                        )
                    nc.gpsimd.dma_start(
                        out_ext.ap()[ni * P : (ni + 1) * P, :], ot[:]
                    )

            def stats_group(g):
                # Per-pair stats: scal[i] becomes ready incrementally, so
                # early strip stts (and the pcs ring behind them) unblock
                # before the whole group's squares finish.
                for i2 in range(TPS * g, TPS * g + TPS, 2):
                    square_tile(i2)
                    square_tile(i2 + 1)
                    stats_range(i2, i2 + 1)

            # Tiles 0-3: load + transpose upfront (nothing else on PE yet).
            # Squares emitted after the x0T copies so the Scalar queue
            # doesn't starve the transposes' consumers.
            for i in range(TPS):
                load_tile(i, split=True)
            mask_load(0)
            pend.extend((i, e) for i in range(TPS) for e in range(DK))
            drip(TPS * DK)
            stats_group(0)
            # Tiles 4-15: loads one strip ahead; transposes dripped 1:1
            # between strip/ctx matmuls of the preceding strip; mask DMAs
            # pre-issued one strip ahead.
            for i in range(TPS, 2 * TPS):
                load_tile(i, split=True)
            pend.extend((i, e) for i in range(TPS, 2 * TPS) for e in range(DK))

            prev_A = None
            for nj in range(NS):
                if nj > 0:
                    mask_load(nj)
                A = phase_a(nj)
                if nj + 1 < NS:
                    stats_group(nj + 1)
                if nj + 2 < NS:
                    for i in range(TPS * (nj + 2), TPS * (nj + 3)):
                        load_tile(i)
                    pend.extend(
                        (i, e)
                        for i in range(TPS * (nj + 2), TPS * (nj + 3))
                        for e in range(DK)
                    )
                if prev_A is not None:
                    phase_b(nj - 1, prev_A)
                prev_A = A
            phase_b(NS - 1, prev_A, last=True)

    nc.compile()
    return nc


def _get_nc():
    if "nc" not in _NC_CACHE:
        _NC_CACHE["nc"] = _build()
    return _NC_CACHE["nc"]


def _run(x, spatial_proj, trace=False):
    import ml_dtypes
    from concourse.bass_utils import run_bass_kernel_spmd

    nc = _get_nc()
    x = np.asarray(x, dtype=np.float32)
    sp = np.asarray(spatial_proj, dtype=np.float32)
    maskT = np.ascontiguousarray(
        np.tril(sp).T.astype(ml_dtypes.bfloat16)
    )
    x0b = np.ascontiguousarray(x[:, :, 0:D].astype(ml_dtypes.bfloat16))
    x1b = np.ascontiguousarray(x[:, :, D : 2 * D].astype(ml_dtypes.bfloat16))
    in_maps = [
        {"x0b": x0b[b], "x1b": x1b[b], "maskT": maskT} for b in range(B)
    ]
    res = run_bass_kernel_spmd(
        nc, in_maps, core_ids=list(range(B)), trace=trace
    )
    out = np.stack([res.results[b]["out"] for b in range(B)], axis=0)
    return out.astype(np.float32), res


def kernel(x, spatial_proj):
    out, _ = _run(x, spatial_proj, trace=False)
    return out


if __name__ == "__main__":
    rng = np.random.default_rng(0)
    x = rng.standard_normal((B, N, 2 * D), dtype=np.float32)
    sp = (rng.standard_normal((N, N), dtype=np.float32) * np.sqrt(1.0 / N)).astype(
        np.float32
    )
    out = kernel(x, sp)
    print("out shape", out.shape, out.dtype)


# revision 55
# speedup vs baseline: 1.0019x; 1.0019x over previous
"""Trainium2 Bass kernel for nn_Contextualizer (sparse_attention).

Per-core computation (data-parallel over batch B=8 across 8 NeuronCores):
    x0, x1 = split(x, 2, axis=-1)            # [N, D] each, N=2048, D=1024
    xn = x0 / sqrt(sum(x0^2, -1) + eps)      # row-normalize
    cosim = xn @ xn.T                        # [N, N], symmetric
    attn = tril(spatial_proj) * cosim
    out = (attn @ x0) * x1

Gram formulation with both 1/norm factors folded into elementwise stages:
    A[m, n]   = maskT[m, n] * G[m, n] * s[m]     (mask stage, m on partitions)
    out[n, d] = (ctx'[n, d] * s[n]) * x1[n, d]   (gating stage)
where s = 1/sqrt(sum x0^2 + eps), maskT = tril(spatial_proj).T (host-side,
bf16), and A (= attn transposed, via G symmetry) feeds matmul2 as the
stationary operand: ctx'[n, d] = sum_m A[m, n] * x0[m, d].

The tril mask kills the upper triangle: matmul1 strips are 512 wide with
partial-width matmuls at the diagonal (exact triangular work), matmul2
accumulates only tiles m <= n.  PE transposes of x0 (for the Gram moving
operand) are dripped 1:1 between strip/ctx matmuls so their identity
LDWEIGHTS hide under the 213ns matmuls.
"""

import numpy as np

B = 8
N = 2048
D = 1024
P = 128
NT = N // P      # 16 row tiles
DK = D // P      # 8 contraction chunks for matmul1
SW = 512         # matmul1 strip width (n)
NS = N // SW     # 4 strips
TPS = SW // P    # 4 row tiles per strip
EPS = 1e-8

_NC_CACHE = {}


def _build():
    from concourse import bacc, mybir
    from concourse.tile import TileContext
    from concourse.masks import make_identity

    f32 = mybir.dt.float32
    f32r = mybir.dt.float32r
    bf16 = mybir.dt.bfloat16
    AF = mybir.ActivationFunctionType
    OP = mybir.AluOpType

    nc = bacc.Bacc("TRN2", debug=False, num_devices=B)
    x0_ext = nc.declare_dram_parameter("x0b", [N, D], bf16, isOutput=False)
    x1_ext = nc.declare_dram_parameter("x1b", [N, D], bf16, isOutput=False)
    m_ext = nc.declare_dram_parameter("maskT", [N, N], bf16, isOutput=False)
    out_ext = nc.declare_dram_parameter("out", [N, D], bf16, isOutput=True)

    with TileContext(nc) as tc:
        with (
            tc.tile_pool(name="big", bufs=1) as big,
            tc.tile_pool(name="sqp", bufs=2) as sqp,
            tc.tile_pool(name="astrip", bufs=2) as astrip,
            tc.tile_pool(name="maskp", bufs=14) as maskp,
            tc.tile_pool(name="gio", bufs=4) as gio,
            tc.tile_pool(name="outp", bufs=2) as outp,
            tc.tile_pool(name="pt", bufs=2, space="PSUM") as pt,
            tc.tile_pool(name="pa", bufs=3, space="PSUM") as pa,
            tc.tile_pool(name="pb", bufs=3, space="PSUM") as pb,
        ):
            xb = big.tile([P, NT, D], bf16)    # x0 bf16, natural [m, d]
            x0T = big.tile([P, DK, N], bf16)   # x0 transposed: [d, n]
            ident = big.tile([P, P], bf16)
            make_identity(nc, ident)
            nrm2 = big.tile([P, NT], f32)
            scal = big.tile([P, NT], f32)
            tmpa = big.tile([P, NT], f32)
            tmpb = big.tile([P, NT], f32)

            def load_tile(i, split=False):
                """DMA x0 tile i (bf16, host-cast) straight into xb."""
                src = x0_ext.ap()[i * P : (i + 1) * P, :]
                if split:
                    # Halves on both queues: transposes of e<4 start as soon
                    # as the first half lands.
                    nc.sync.dma_start(xb[:, i, 0:512], src[:, 0:512])
                    nc.scalar.dma_start(xb[:, i, 512:D], src[:, 512:D])
                else:
                    eng = nc.sync if i % 2 == 0 else nc.scalar
                    eng.dma_start(xb[:, i, :], src[:])

            def square_tile(i):
                sq = sqp.tile([P, D], f32, tag="sq")
                nc.scalar.activation(
                    sq[:], xb[:, i, :], AF.Square, accum_out=nrm2[:, i : i + 1]
                )

            def stats_range(i0, i1):
                """scal[:, i0:i1+1] = rsqrt(nrm2 + EPS), one Newton step."""
                sl = slice(i0, i1 + 1)
                nc.gpsimd.tensor_scalar_add(tmpa[:, sl], nrm2[:, sl], EPS)
                nc.scalar.activation(tmpb[:, sl], tmpa[:, sl], AF.Sqrt)
                nc.vector.reciprocal(scal[:, sl], tmpb[:, sl])
                nc.gpsimd.tensor_mul(tmpb[:, sl], scal[:, sl], scal[:, sl])
                nc.gpsimd.tensor_mul(tmpb[:, sl], tmpb[:, sl], tmpa[:, sl])
                nc.gpsimd.tensor_scalar(
                    tmpb[:, sl], tmpb[:, sl], -0.5, 1.5, op0=OP.mult, op1=OP.add
                )
                nc.gpsimd.tensor_mul(scal[:, sl], scal[:, sl], tmpb[:, sl])

            # Pending PE-transpose drip queue: (tile, e) pairs in order.
            pend = []
            pt_cur = [None]

            def drip(k):
                """Emit up to k pending transpose matmuls (+copy at tile end)."""
                for _ in range(k):
                    if not pend:
                        return
                    i, e = pend.pop(0)
                    if e == 0:
                        pt_cur[0] = pt.tile(
                            [P, DK, P], bf16, tag="ps", name="ps"
                        )
                    ps = pt_cur[0]
                    nc.tensor.transpose(
                        ps[:, e, :], xb[:, i, e * P : (e + 1) * P], ident
                    )
                    if e == DK - 1:
                        nc.scalar.copy(x0T[:, :, i * P : (i + 1) * P], ps[:])

            mask_rr = [0]
            mask_q = [None] * NS

            def mask_load(nj):
                """Issue strip nj's mask DMAs (sync queue, ahead of use)."""
                n0 = nj * SW
                mts = []
                for mi2 in range(0, TPS * nj + TPS, 2):
                    mt = maskp.tile([P, 2, SW], bf16, tag="mt", name="mt")
                    # Late strips alternate sync/scalar so mask transfers
                    # stop queueing ahead of tile 8-15 loads on sync.
                    if nj >= 2:
                        eng = nc.sync if mask_rr[0] % 2 == 0 else nc.scalar
                    else:
                        eng = nc.sync
                    mask_rr[0] += 1
                    eng.dma_start(
                        mt[:],
                        m_ext.ap()[
                            mi2 * P : (mi2 + 2) * P, n0 : n0 + SW
                        ].rearrange("(c p) n -> p c n", p=P),
                    )
                    mts.append(mt)
                mask_q[nj] = mts

            def phase_a(nj):
                """A strip for n in [SW*nj, SW*nj+SW): m-tiles 0..4nj+3."""
                n0 = nj * SW
                n_mtiles = TPS * nj + TPS
                A = astrip.tile([P, NT, SW], bf16, tag="A")
                for mi2 in range(0, n_mtiles, 2):
                    mt = mask_q[nj][mi2 // 2]
                    for c in range(2):
                        mi = mi2 + c
                        off = max(0, (mi - TPS * nj) * P)
                        pcs = pa.tile([P, SW], f32, tag="pcs")
                        for e in range(DK):
                            nc.tensor.matmul(
                                pcs[:, off:SW],
                                x0T[:, e, mi * P : (mi + 1) * P],
                                x0T[:, e, n0 + off : n0 + SW],
                                start=(e == 0),
                                stop=(e == DK - 1),
                            )
                            # Defer dripping past the first pair so the next
                            # group's tiles have landed before their
                            # transposes hit the in-order PE queue.
                            if mi2 > 0:
                                drip(2)
                        nc.vector.scalar_tensor_tensor(
                            out=A[:, mi, off:SW],
                            in0=pcs[:, off:SW],
                            scalar=scal[:, mi : mi + 1],
                            in1=mt[:, c, off:SW],
                            op0=OP.mult,
                            op1=OP.mult,
                        )
                return A

            def phase_b(nj, A, last=False):
                """ctx rows for n-tiles 4nj..4nj+3; scale+gate; DMA out."""
                x1ts = []
                for sub in range(TPS):
                    ni = TPS * nj + sub
                    x1t = gio.tile([P, D], bf16, tag="x1t")
                    nc.gpsimd.dma_start(
                        x1t[:], x1_ext.ap()[ni * P : (ni + 1) * P, :]
                    )
                    x1ts.append(x1t)
                for sub in range(TPS):
                    ni = TPS * nj + sub
                    x1t = x1ts[sub]
                    ot = outp.tile([P, D], bf16, tag="ot")
                    if last and sub == TPS - 1:
                        # Tail: quarter-width PSUM groups so gating + the
                        # final store overlap the last matmuls.
                        for q in range(4):
                            pq = pb.tile([P, 512], f32, tag="pob", name="pob")
                            for mi in range(ni + 1):
                                nc.tensor.matmul(
                                    pq[:, 0:256],
                                    A[:, mi, sub * P : (sub + 1) * P],
                                    xb[:, mi, q * 256 : (q + 1) * 256],
                                    start=(mi == 0),
                                    stop=(mi == ni),
                                )
                            c0 = q * 256
                            nc.vector.scalar_tensor_tensor(
                                out=ot[:, c0 : c0 + 256],
                                in0=pq[:, 0:256],
                                scalar=scal[:, ni : ni + 1],
                                in1=x1t[:, c0 : c0 + 256],
                                op0=OP.mult,
                                op1=OP.mult,
                            )
                            eng = nc.scalar if q % 2 == 0 else nc.sync
                            eng.dma_start(
                                out_ext.ap()[
                                    ni * P : (ni + 1) * P, c0 : c0 + 256
                                ],
                                ot[:, c0 : c0 + 256],
                            )
                        continue
                    pob0 = pb.tile([P, 512], f32, tag="pob")
                    pob1 = pb.tile([P, 512], f32, tag="pob")
                    for mi in range(ni + 1):
                        st = A[:, mi, sub * P : (sub + 1) * P]
                        nc.tensor.matmul(
                            pob0[:], st, xb[:, mi, 0:512],
                            start=(mi == 0), stop=(mi == ni),
                        )
                        nc.tensor.matmul(
                            pob1[:], st, xb[:, mi, 512:D],
                            start=(mi == 0), stop=(mi == ni),
                        )
                        drip(1)
                    for dc, pob in enumerate((pob0, pob1)):
                        nc.vector.scalar_tensor_tensor(
                            out=ot[:, dc * 512 : (dc + 1) * 512],
                            in0=pob[:],
                            scalar=scal[:, ni : ni + 1],
                            in1=x1t[:, dc * 512 : (dc + 1) * 512],
                            op0=OP.mult,
                            op1=OP.mult,
                        )
                    nc.gpsimd.dma_start(
                        out_ext.ap()[ni * P : (ni + 1) * P, :], ot[:]
                    )

            def stats_group(g):
                # Per-pair stats: scal[i] becomes ready incrementally, so
                # early strip stts (and the pcs ring behind them) unblock
                # before the whole group's squares finish.
                for i2 in range(TPS * g, TPS * g + TPS, 2):
                    square_tile(i2)
                    square_tile(i2 + 1)
                    stats_range(i2, i2 + 1)

            # Tiles 0-3: load + transpose upfront (nothing else on PE yet).
            # Squares emitted after the x0T copies so the Scalar queue
            # doesn't starve the transposes' consumers.
            for i in range(TPS):
                load_tile(i, split=True)
            mask_load(0)
            pend.extend((i, e) for i in range(TPS) for e in range(DK))
            drip(TPS * DK)
            stats_group(0)
            # Tiles 4-15: loads one strip ahead; transposes dripped 1:1
            # between strip/ctx matmuls of the preceding strip; mask DMAs
            # pre-issued one strip ahead.
            for i in range(TPS, 2 * TPS):
                load_tile(i, split=True)
            pend.extend((i, e) for i in range(TPS, 2 * TPS) for e in range(DK))

            prev_A = None
            for nj in range(NS):
                if nj > 0:
                    mask_load(nj)
                A = phase_a(nj)
                if nj + 1 < NS:
                    stats_group(nj + 1)
                if nj + 2 < NS:
                    for i in range(TPS * (nj + 2), TPS * (nj + 3)):
                        load_tile(i)
                    pend.extend(
                        (i, e)
                        for i in range(TPS * (nj + 2), TPS * (nj + 3))
                        for e in range(DK)
                    )
                if prev_A is not None:
                    phase_b(nj - 1, prev_A)
                prev_A = A
            phase_b(NS - 1, prev_A, last=True)

    nc.compile()
    return nc


def _get_nc():
    if "nc" not in _NC_CACHE:
        _NC_CACHE["nc"] = _build()
    return _NC_CACHE["nc"]


def _run(x, spatial_proj, trace=False):
    import ml_dtypes
    from concourse.bass_utils import run_bass_kernel_spmd

    nc = _get_nc()
    x = np.asarray(x, dtype=np.float32)
    sp = np.asarray(spatial_proj, dtype=np.float32)
    maskT = np.ascontiguousarray(
        np.tril(sp).T.astype(ml_dtypes.bfloat16)
    )
    x0b = np.ascontiguousarray(x[:, :, 0:D].astype(ml_dtypes.bfloat16))
    x1b = np.ascontiguousarray(x[:, :, D : 2 * D].astype(ml_dtypes.bfloat16))
    in_maps = [
        {"x0b": x0b[b], "x1b": x1b[b], "maskT": maskT} for b in range(B)
    ]
    res = run_bass_kernel_spmd(
        nc, in_maps, core_ids=list(range(B)), trace=trace
    )
    out = np.stack([res.results[b]["out"] for b in range(B)], axis=0)
    return out.astype(np.float32), res


def kernel(x, spatial_proj):
    out, _ = _run(x, spatial_proj, trace=False)
    return out


if __name__ == "__main__":
    rng = np.random.default_rng(0)
    x = rng.standard_normal((B, N, 2 * D), dtype=np.float32)
    sp = (rng.standard_normal((N, N), dtype=np.float32) * np.sqrt(1.0 / N)).astype(
        np.float32
    )
    out = kernel(x, sp)
    print("out shape", out.shape, out.dtype)
